# revision 74
# baseline (speedup 1.0000x reference)
"""Trainium2 Bass kernel for windowed multi-lag autocorrelation.

Reference computation (per (batch, seq) row of x[16, 128, 8320]):
  - 64 overlapping windows of length 256, stride 128
  - per-window mean removal, hanning window
  - autocorrelation at lags 0..31, scaled by 1/256
  -> out [16, 128, 1, 64, 32]

Device formulation (quadratic op -> DFT trick so the PE does the work):
  autocorr(w)[a] = (1/N) sum_f alpha_f |DFT_N(w)|^2[f] * cos(2*pi*f*a/N)
  with N = 255 (odd -> rfft bins f=0..127 fill the 128 partitions exactly).
  N < 256+32 makes the transform circular; the aliased lags 224..255 only
  touch hanning-damped window edges (~1e-4 rel err measured).
  Mean removal + hanning fold into the forward matrices.

Mixed precision (v2):
  - cos path: bf16 operands (x bf16 stream, A_cos bf16), 2 matmuls/group.
  - sin path: fp8 e4m3 (x8 stream, A_sin fp8) via ONE DoubleRow matmul
    contracting K=256 (both window chunks as the two k-tiles; the k-stride
    in the moving AP is just the 8-column chunk offset of the layout).
    DoubleRow measures ~1.4x over the two bf16 matmuls it replaces.
  - B is split per path (the inverse accumulates cos^2 and sin^2 in
    separate matmuls anyway): B_sin is least-squares re-fit against the
    QUANTIZED A_sin so the fp8 A error is partially compensated for free.
    Simulated end-to-end rel_l2 ~1.5e-2 vs the 2e-2 gate.

Per group of 8 rows (512 windows, free-dim column n = chunk*8 + row so
both window halves are stride-1 slices xv[:,0:512] / xv[:,8:520]):
  PE:    2 bf16 matmuls (cos, PSUM-accumulated) + 1 fp8 DR matmul (sin)
         + 8 transposed inverse matmuls: stationary sq[:, 128k:128k+128]
         (cos^2 with b_cos then sin^2 with b_sin, PSUM-accumulated),
         moving B [128f, 32lags] -> out [128 windows, 32 lags].
         Steady-state measured: cos 2x215ns, sin DR ~250ns, inverse
         8x25ns (LDWEIGHTS-bound) -> ~900ns/group, the cadence floor.
  Act:   sq_c = square(ps_cos) (PSUM -> bf16 SBUF; Act is the only engine
         that squares straight out of PSUM) + half the sblock out-copy
         + every 8th sin^2 as square(s_sb) from SBUF (~450ns).
  DVE:   s_sb = copy(ps_sin) bf16 + other half of the out-copy + every
         8th sin^2 mul.
  GpSimd:sq_s = s_sb * s_sb for 6 of 8 groups (it is the slow engine,
         ~1051ns per [128,512] TT).
  The inverse of group g-4 issues after group g's forwards (lag-4 software
  pipelining, tapering to lag-2 near the end so the drain doesn't all
  serialize after the last forward); 4 groups' inverse outputs share one
  [128, 512] PSUM bank. cos and sin PSUM live in separate pools so Act
  and DVE free their tiles independently.

DMA: queues must be single-purpose (sharing one drops ~270GB/s to ~120):
bf16 input on sync, fp8 input on gpsimd (fewer/bigger chunks since its
per-partition runs are 1B/col), output on the scalar queue batched as
{0-3}, {4-6}, {7} with the final sblock split by partition halves across
two queues (DRAM writes pay ~20ns/packet, so big runs and a small last
transfer matter).

Sharding: pure data parallel, 2 batches per core across 8 cores.
"""
import os

# must be set before NRT initializes: recovers cores left wedged by a
# previous crashed run (NRT_EXEC_UNIT_UNRECOVERABLE otherwise)
os.environ.setdefault("NEURON_RT_RESET_CORES", "1")

import numpy as np
import ml_dtypes

import concourse.bass as bass
import concourse.tile as tile
from concourse import mybir
from concourse.bass import AP
from concourse.bass_utils import run_bass_kernel_spmd

NUM_AUTOCORR = 32
NUM_WINDOWS = 64
WIN_LEN = 256
WIN_STRIDE = 128
NFFT = 255
NF = 128  # rfft bins 0..127 (N odd)
SEQ = 128
BATCH = 16
VALUE = (NUM_WINDOWS - 1) * WIN_STRIDE + WIN_LEN  # 8320
NCHUNK = VALUE // WIN_STRIDE  # 65
N_CORES = 8
ROWS_PER_CORE = (BATCH // N_CORES) * SEQ  # 256
G = 8  # rows per group
NGROUP = ROWS_PER_CORE // G  # 32
NW = G * NUM_WINDOWS  # 512 windows per group (matmul free dim)
GW = G * NCHUNK  # 520 columns per group in the input tile
CB_COL = 2 * 128 + 2 * NUM_AUTOCORR  # bf16 const cols (Ac1|Ac2|Bcos|Bsin)
C8_COL = 256  # fp8 const cols (A_sin k-tiles side by side)
SB = 4  # groups stacked per output super-block (PSUM partition offsets)
NSB = NGROUP // SB  # 8
# progressive input DMA chunking: each dma_start costs ~565ns on the issuing
# engine, so few big issues beat many small ones; chunk 0 carries the consts
# so the PE can start after one transfer per stream. Later chunks grow so
# the per-partition DMA runs get bigger (higher effective bandwidth).
IN_CHUNKS = [1, 2, 2, 3, 4, 5, 7, 8]
assert sum(IN_CHUNKS) == NGROUP
# the fp8 stream is 1B/col, so runs are half the bf16 stream's; use fewer,
# bigger chunks to keep its effective DMA bandwidth up
IN_CHUNKS8 = [1, 3, 4, 6, 9, 9]
assert sum(IN_CHUNKS8) == NGROUP

F32 = mybir.dt.float32
BF16 = mybir.dt.bfloat16
FP8 = mybir.dt.float8e4
U8 = mybir.dt.uint8
NP_FP8 = ml_dtypes.float8_e4m3  # TRN float8e4 flavor (max 240)
N_WARMUP = 6  # dummy matmuls to ramp the PE clock while input DMA runs

LAST_EXEC_NS = None


def _build_mats():
    i = np.arange(WIN_LEN)
    f = np.arange(NF)
    h = np.hanning(WIN_LEN)
    ang = 2 * np.pi * np.outer(i, f) / NFFT
    C = h[:, None] * np.cos(ang)
    S = h[:, None] * np.sin(ang)
    Sb = np.zeros_like(S)
    Sb[:, 1:] = S[:, 1:]  # sin col j holds bin f=j; col 0 is a zero pad
    Ac = C - C.mean(axis=0, keepdims=True)  # fold per-window mean removal
    As = Sb - Sb.mean(axis=0, keepdims=True)
    fa = 2 * np.pi * np.outer(f, np.arange(NUM_AUTOCORR)) / NFFT
    alpha = np.full(NF, 2.0)
    alpha[0] = 1.0
    B = alpha[:, None] * np.cos(fa) / (NFFT * WIN_LEN)

    As8 = As.astype(NP_FP8)
    As8f = As8.astype(np.float32)
    # least-squares re-fit of B_sin against the quantized A_sin: choose
    # per-bin weights D so sum_f D[f] a8_f a8_f^T best matches the exact
    # quadratic form sum_f B[f] a_f a_f^T (Frobenius LS via the Gram matrix)
    Gm = (As8f.T @ As8f) ** 2
    M = (As8f.T @ As) ** 2
    Bs = np.linalg.lstsq(Gm + 1e-9 * np.eye(NF), M @ B, rcond=None)[0]
    return (
        Ac.astype(np.float32),
        As8,
        B.astype(np.float32),
        Bs.astype(np.float32),
    )


def _split_sync_waits(nc, max_waits=1):
    """walrus in this container rejects instructions with multiple sem waits
    ("Too many sync wait commands"); split extras into single-wait NoOps."""
    ctr = [0]

    def mknop(engine, waits):
        ctr[0] += 1
        nop = mybir.InstNoOp(name=f"waitsplit-{ctr[0]}", ins=[], outs=[])
        nop.engine = engine
        nop.sync_info = mybir.SyncInfo(on_wait=list(waits), on_update=[])
        return nop

    for fn in nc.m.functions:
        for blk in fn.blocks:
            out = []
            changed = False
            for inst in blk.instructions:
                si = inst.sync_info
                waits = list(si.on_wait) if si is not None and si.on_wait else []
                if len(waits) > max_waits:
                    changed = True
                    extra, keep = waits[:-max_waits], waits[-max_waits:]
                    for k in range(0, len(extra), max_waits):
                        out.append(mknop(inst.engine, extra[k : k + max_waits]))
                    inst.sync_info = mybir.SyncInfo(
                        on_wait=keep, on_update=list(si.on_update or [])
                    )
                out.append(inst)
            if changed:
                blk.instructions = out
    return nc


def _build_kernel():
    nc = bass.Bass(target_bir_lowering=False)
    # xtb[p, CB_COL + g*520 + c*8 + r] = x[row 8g+r, 128c + p] in bf16;
    # xt8 same layout in fp8 (C8_COL const prefix). Any column-range DMA
    # slice is per-partition contiguous in DRAM.
    xtb = nc.dram_tensor("xtb", [128, CB_COL + NGROUP * GW], BF16, kind="ExternalInput")
    xt8 = nc.dram_tensor("xt8", [128, C8_COL + NGROUP * GW], FP8, kind="ExternalInput")
    # partition-major output: adjacent sblocks are column-adjacent, so a
    # two-sblock DMA gets 2KB per-partition runs (DRAM writes at 1KB run
    # ~40GB/s; bigger runs matter)
    out = nc.dram_tensor("out", [128, NSB * NW], BF16, kind="ExternalOutput")

    with tile.TileContext(nc) as tc:
        with (
            tc.tile_pool(name="xinb", bufs=1) as xbpool,
            tc.tile_pool(name="sqp", bufs=10) as sqpool,
            tc.tile_pool(name="ssb", bufs=10) as spool,
            # one SBUF tile per sblock: the output DMAs ride the sync queue
            # behind the bulk input, so the staging tiles must stay live
            # until ~2/3 through the run
            tc.tile_pool(name="outb", bufs=NSB) as opool,
            # separate cos/sin PSUM pools: Act frees a cos tile without
            # waiting for DVE's sin drain (finer-grained dependencies)
            tc.tile_pool(name="psc", bufs=3, space="PSUM") as pscpool,
            tc.tile_pool(name="pss", bufs=3, space="PSUM") as psspool,
            tc.tile_pool(name="pso", bufs=2, space="PSUM") as psopool,
        ):
            # warm-tile memset FIRST on gpsimd (it exits the NEFF preamble
            # earliest) so the PE clock-ramp warmup starts as soon as the
            # tensor engine's own preamble ends
            warm = xbpool.tile([128, 128 + NW], BF16, tag="warm")
            nc.gpsimd.memset(warm[:], 0)

            # each stream gets its own clean queue (a queue shared with
            # another stream or with output drops from ~270GB/s to ~120)
            bchunks = []  # (tile, first_group, n_groups, col_offset)
            chunks8 = []
            g0 = 0
            for ci, sz in enumerate(IN_CHUNKS):
                colsb = sz * GW + (CB_COL if ci == 0 else 0)
                xb_t = xbpool.tile([128, colsb], BF16, tag=f"xb{ci}")
                lob = 0 if ci == 0 else CB_COL + g0 * GW
                nc.sync.dma_start(xb_t[:], xtb.ap()[:, lob : lob + colsb])
                bchunks.append((xb_t, g0, sz, CB_COL if ci == 0 else 0))
                g0 += sz
            g0 = 0
            for ci, sz in enumerate(IN_CHUNKS8):
                cols8 = sz * GW + (C8_COL if ci == 0 else 0)
                x8_t = xbpool.tile([128, cols8], FP8, tag=f"x8{ci}")
                lo8 = 0 if ci == 0 else C8_COL + g0 * GW
                nc.gpsimd.dma_start(x8_t[:], xt8.ap()[:, lo8 : lo8 + cols8])
                chunks8.append((x8_t, g0, sz, C8_COL if ci == 0 else 0))
                g0 += sz

            # PE p-state ramps 0.65 -> 2.4 GHz over ~3us of busy time; burn
            # the input-DMA wait on dummy matmuls over the memset tile so
            # real work starts at full clock.
            ps_warm = pscpool.tile([128, NW], F32, tag="ps_c")
            for _ in range(N_WARMUP):
                nc.tensor.matmul(
                    ps_warm[:], warm[:, 0:128], warm[:, 128 : 128 + NW],
                    start=True, stop=True,
                )

            cb0 = bchunks[0][0]
            a_c1 = cb0[:, 0:128]
            a_c2 = cb0[:, 128:256]
            b_cos = cb0[:, 256 : 256 + NUM_AUTOCORR]
            b_sin = cb0[:, 256 + NUM_AUTOCORR : 256 + 2 * NUM_AUTOCORR]
            # stationary for the sin DoubleRow matmul: [128p, 2k, 128m]
            # over the [128, 256] fp8 const block (k-tile stride 128 cols)
            c80 = chunks8[0][0][:, 0:C8_COL]
            as8_st = AP(c80.tensor, c80.offset,
                        [list(c80.ap[0]), [128, 2], [1, 128]])

            def group_view(g):
                """-> (bf16 view [128, 520], fp8 view [128, 520])"""
                bv = fv = None
                for t, gg0, sz, off in bchunks:
                    if gg0 <= g < gg0 + sz:
                        bv = t[:, off + (g - gg0) * GW : off + (g - gg0) * GW + GW]
                for t, gg0, sz, off in chunks8:
                    if gg0 <= g < gg0 + sz:
                        fv = t[:, off + (g - gg0) * GW : off + (g - gg0) * GW + GW]
                assert bv is not None and fv is not None
                return bv, fv

            # lag-4 software pipeline: group g's inverse issues after group
            # g+4's forward, hiding the square-chain latency from the PE
            pend = []  # [(g, sq), ...]
            psout_t = None
            o_pair = None

            def flush_inverse():
                nonlocal pend, psout_t, o_pair
                if not pend:
                    return
                g, sq = pend.pop(0)
                j = g % SB
                if j == 0:
                    psout_t = psopool.tile([128, NW], F32, tag="ps_out")
                for k in range(4):
                    lo = 128 * j + 32 * k
                    nc.tensor.matmul(
                        psout_t[:, lo : lo + 32],
                        sq[:, 128 * k : 128 * k + 128], b_cos,
                        start=True, stop=False,
                    )
                    nc.tensor.matmul(
                        psout_t[:, lo : lo + 32],
                        sq[:, NW + 128 * k : NW + 128 * k + 128], b_sin,
                        start=False, stop=True,
                    )
                sblk = g // SB
                # output staging batches: sblocks {0..3} in one [128, 2048]
                # tile (4KB per-partition DMA runs), {4..6} in a [128, 1536]
                # tile, {7} alone. DRAM writes pay ~20ns/packet, so few DMAs
                # with big runs beat per-sblock 1KB-run transfers by 3-4x.
                if sblk < NSB - 1:
                    if j != SB - 1:
                        return
                    if sblk == 0:
                        o_pair = opool.tile([128, 4 * NW], BF16, tag="o03")
                    elif sblk == 4:
                        o_pair = opool.tile([128, 3 * NW], BF16, tag="o46")
                    po = (sblk % 4 if sblk < 4 else sblk - 4) * NW
                    # copy split between Act and DVE halves so neither drain
                    # engine eats the whole 687ns PSUM read
                    nc.scalar.copy(
                        o_pair[:, po : po + NW // 2], psout_t[:, 0 : NW // 2]
                    )
                    nc.vector.tensor_copy(
                        o_pair[:, po + NW // 2 : po + NW], psout_t[:, NW // 2 : NW]
                    )
                    if sblk == 3:
                        # scalar queue carries nothing else (sharing a queue
                        # with input tanks both streams' bandwidth)
                        nc.scalar.dma_start(out.ap()[:, 0 : 4 * NW], o_pair[:])
                    elif sblk == 6:
                        nc.scalar.dma_start(
                            out.ap()[:, 4 * NW : 7 * NW], o_pair[:]
                        )
                else:
                    # final sblock: copies per inverse-group piece (engines
                    # are idle by now), one DMA right after the last copy
                    if j == 0:
                        o_pair = opool.tile([128, NW], BF16, tag="o7")
                    lo = 128 * j
                    if j % 2 == 0:
                        nc.scalar.copy(
                            o_pair[:, lo : lo + 128], psout_t[:, lo : lo + 128]
                        )
                    else:
                        nc.vector.tensor_copy(
                            o_pair[:, lo : lo + 128], psout_t[:, lo : lo + 128]
                        )
                    if j == SB - 1:
                        # split by partition halves across two queues: each
                        # half is 64 x 1KB packets, halving the ~2.5us tail
                        nc.scalar.dma_start(
                            out.ap()[0:64, 7 * NW : 8 * NW], o_pair[0:64, :]
                        )
                        nc.sync.dma_start(
                            out.ap()[64:128, 7 * NW : 8 * NW], o_pair[64:128, :]
                        )

            for g in range(NGROUP):
                xvb, xv8 = group_view(g)
                ps_s = psspool.tile([128, NW], F32, tag="ps_s")
                ps_c = pscpool.tile([128, NW], F32, tag="ps_c")
                # cos first: Act's square unblocks ~450ns earlier, and the
                # sin path's fp8 chunk gets that much more delivery slack
                nc.tensor.matmul(
                    ps_c[:], a_c1, xvb[:, 0:NW], start=True, stop=False
                )
                nc.tensor.matmul(
                    ps_c[:], a_c2, xvb[:, G : G + NW], start=False, stop=True
                )
                mov8 = AP(xv8.tensor, xv8.offset,
                          [list(xv8.ap[0]), [G, 2], [1, NW]])
                nc.tensor.matmul(
                    ps_s[:], as8_st, mov8, start=True, stop=True,
                    perf_mode=mybir.MatmulPerfMode.DoubleRow,
                )

                if len(pend) >= 4:
                    flush_inverse()
                    # taper: drain the lag pipeline early so the last
                    # groups' inverses don't all serialize after the loop
                    if g >= NGROUP - 5:
                        flush_inverse()

                sq = sqpool.tile([128, 2 * NW], BF16, tag="sq")
                nc.scalar.square(sq[:, 0:NW], ps_c[:])
                s_sb = spool.tile([128, NW], BF16, tag="s_sb")
                nc.vector.tensor_copy(s_sb[:], ps_s[:])
                # sin^2: gpsimd is ~1051ns per 512-wide TT, over the target
                # cadence, so Act takes every 4th group via square(s_sb)
                # (one-input ACTIVATE from SBUF); in the last 4 groups
                # alternate DVE/gpsimd so the drain tail pipelines
                if g >= NGROUP - 4:
                    mul_eng = nc.gpsimd if g % 2 == 0 else nc.vector
                    mul_eng.tensor_mul(sq[:, NW : 2 * NW], s_sb[:], s_sb[:])
                elif g % 8 == 3:
                    nc.scalar.square(sq[:, NW : 2 * NW], s_sb[:])
                elif g % 8 == 7:
                    mul_eng = nc.vector
                    mul_eng.tensor_mul(sq[:, NW : 2 * NW], s_sb[:], s_sb[:])
                else:
                    nc.gpsimd.tensor_mul(sq[:, NW : 2 * NW], s_sb[:], s_sb[:])
                pend.append((g, sq))

            while pend:
                flush_inverse()

    _split_sync_waits(nc)
    return nc


def _install_ntff_shim():
    """The trimmed antenv lacks axon_hooks, so trace=True degrades to no
    profile. Recreate the hook: ctypes into libaxon_pjrt.so (same ABI the
    boot shim uses), exposed as a synthetic antenv.axon_hooks module."""
    import sys
    import ctypes
    import contextlib
    import types

    if "antenv.axon_hooks" in sys.modules:
        return
    so_path = "/opt/axon/libaxon_pjrt.so"
    if not os.path.exists(so_path):
        return
    lib = ctypes.CDLL(so_path)
    if not hasattr(lib, "axon_start_nrt_profile"):
        return
    lib.axon_start_nrt_profile.argtypes = [
        ctypes.POINTER(ctypes.c_int64),
        ctypes.c_size_t,
    ]
    lib.axon_start_nrt_profile.restype = ctypes.c_int64
    lib.axon_stop_nrt_profile.argtypes = [ctypes.c_char_p]
    lib.axon_stop_nrt_profile.restype = ctypes.c_int64

    @contextlib.contextmanager
    def _hook(output_dir, device_ids):
        import jax

        jax.devices()
        if device_ids:
            ids = (ctypes.c_int64 * len(device_ids))(*device_ids)
            rc = lib.axon_start_nrt_profile(ids, len(device_ids))
        else:
            rc = lib.axon_start_nrt_profile(None, 0)
        if rc != 0:
            raise RuntimeError(f"axon_start_nrt_profile rc={rc}")
        try:
            yield
        finally:
            n = lib.axon_stop_nrt_profile(str(output_dir).encode())
            print(f"ntff profile: {n} file(s) -> {output_dir}")

    mod = types.ModuleType("antenv.axon_hooks")
    mod.get_axon_ntff_profile_hook = lambda: _hook
    mod.set_axon_ntff_profile_hook = lambda h: None
    sys.modules["antenv.axon_hooks"] = mod

    # avoid network-dependent artifact uploads in the trace path
    import concourse.bass_utils as bu

    bu.upload_artifacts = lambda tmpdir: f"local://{tmpdir}"


_NC_CACHE = None


def _get_nc():
    global _NC_CACHE
    if _NC_CACHE is None:
        _NC_CACHE = _build_kernel()
    return _NC_CACHE


def kernel(x: np.ndarray) -> np.ndarray:
    global LAST_EXEC_NS
    x = np.ascontiguousarray(np.asarray(x), dtype=np.float32)
    assert x.shape == (BATCH, SEQ, VALUE)

    Ac, As8, Bc, Bs = _build_mats()
    constsb = np.zeros((128, CB_COL), np.float32)
    constsb[:, 0:128] = Ac[0:128]
    constsb[:, 128:256] = Ac[128:256]
    constsb[:, 256 : 256 + NUM_AUTOCORR] = Bc
    constsb[:, 256 + NUM_AUTOCORR : 256 + 2 * NUM_AUTOCORR] = Bs
    constsb = constsb.astype(ml_dtypes.bfloat16)
    # fp8 consts: A_sin k-tiles side by side ([p, k*128+m] = As[k*128+p, m])
    consts8 = np.zeros((128, C8_COL), NP_FP8)
    consts8[:, 0:128] = As8[0:128]
    consts8[:, 128:256] = As8[128:256]

    bpc = BATCH // N_CORES
    in_maps = []
    for c in range(N_CORES):
        xc = x[c * bpc : (c + 1) * bpc]  # [2, 128, 8320]
        # xd[p, g, c, r] = x[row 8g+r, 128c + p]
        xd = (
            xc.reshape(NGROUP, G, NCHUNK, WIN_STRIDE)  # [g, r, c, p]
            .transpose(3, 0, 2, 1)  # [p, g, c, r]
            .reshape(128, NGROUP * GW)
        )
        xtb = np.concatenate([constsb, xd.astype(ml_dtypes.bfloat16)], axis=1)
        xt8 = np.concatenate([consts8, xd.astype(NP_FP8)], axis=1)
        in_maps.append(
            {"xtb": np.ascontiguousarray(xtb), "xt8": np.ascontiguousarray(xt8)}
        )

    nc = _get_nc()
    trace = os.environ.get("AUTOCORR_TRACE", "0") == "1"
    if trace:
        _install_ntff_shim()
    try:
        res = run_bass_kernel_spmd(
            nc, in_maps, core_ids=list(range(N_CORES)), trace=trace
        )
    except Exception:
        # a stale/wedged device occasionally fails the first exec after a
        # fresh NEFF load; one retry has always recovered it
        res = run_bass_kernel_spmd(
            nc, in_maps, core_ids=list(range(N_CORES)), trace=trace
        )
    LAST_EXEC_NS = res.exec_time_ns

    outs = []
    for c in range(N_CORES):
        o = np.asarray(res.results[c]["out"]).astype(np.float32)
        # out[p, s*NW + 128j+32k+a] where window-in-group c*8+r = 128k+p,
        # i.e. w = 16k + p//8, r = p%8, group = 4s+j
        o = o.reshape(16, G, NSB, SB, 4, NUM_AUTOCORR)  # [pq, rp, s, j, k, a]
        o = o.transpose(2, 3, 1, 4, 0, 5)  # [s, j, rp, k, pq, a]
        outs.append(o.reshape(bpc, SEQ, NUM_WINDOWS, NUM_AUTOCORR))
    full = np.concatenate(outs, axis=0)  # [16, 128, 64, 32]
    return np.ascontiguousarray(full[:, :, None, :, :])
